# revision 21
# baseline (speedup 1.0000x reference)
"""Trainium2 Bass kernel for nn_GAT (GATv2 x2 + JumpingKnowledge + MLP head).

Self-contained: hardcodes shapes/sharding for the nn_GAT_26757646254515
problem (N=50000 nodes, E=800000 edges, F=64, H=4 heads, 2 GAT passes,
8 NeuronCores).

Sharding: nodes are range-partitioned across 8 cores (6250 destination
nodes per core); each core owns the incoming edges of its nodes (host
pre-sorts edges by destination and groups them into 127-node
destination blocks). Per layer each core transforms its own node shard
(xl_own/xr_own) and the full source table xl_t is assembled by an
AllGather of the shards. Per edge tile (128 edges) the source rows are
fetched with one indirect DMA; the destination rows never move through
DMA: a one-hot selection matrix S_aug ([128 edges x 128], columns =
127 block-local destinations plus the edge attribute) is transposed on
the tensor engine and multiplied against [xr_block ; We] so PSUM
receives xr_dst + ea*We directly. The GATv2 attention then runs in
fp16 slabs of 4 tiles, and the per-destination softmax-weighted sums
accumulate via one-hot matmuls in fp32 PSUM. The global mean pool is
an AllReduce of per-core column sums; the tiny MLP head runs
replicated on every core.
"""

import math

import numpy as np

import concourse.bass as bass
import concourse.mybir as mybir
import concourse.tile as tile
from concourse.tile import ScopedClock

F32 = mybir.dt.float32
F16 = mybir.dt.float16
I32 = mybir.dt.int32

P = 128
BW = 127  # destination nodes per block (128th S_aug column carries ea)
G = 4     # edge tiles per compute chunk


# ---------------------------------------------------------------------------
# Workarounds for this container's walrus build: codegen rejects instructions
# carrying more than one sync-wait command.
# ---------------------------------------------------------------------------
def _patched_drain_and_barrier(self, tick_clock, wait_clock):
    probe = self.nc.sync.nop(nofuse=True)
    wait_clock.add_sem_waits(probe.ins, ScopedClock({None: tick_clock.global_clock}))
    si = probe.ins.sync_info
    if si is not None and len(si.on_wait) > 1:
        waits = list(si.on_wait)
        si.on_wait = waits[:1]
        for w in waits[1:]:
            n = self.nc.sync.nop(nofuse=True)
            n.ins.sync_info = type(si)(on_wait=[w], on_update=[])
    self.nc.sync.drain()
    self.nc.all_engine_barrier()
    assert self.sems is not None
    popped = self.nc._tile_sem_poison_stack.pop()
    assert popped is self._sem_poison
    self.nc.clear_and_free_semaphores(list(self.sems.allocated().values()))
    self.nc.all_engine_barrier()


def apply_tile_patch():
    tile.TileContext._drain_and_barrier = _patched_drain_and_barrier


def split_multi_waits(nc, max_waits=1):
    """Hoist extra sync-waits onto fresh same-engine NoOps inserted
    immediately before the instruction (engines execute serially, so the
    ordering semantics are identical)."""
    import bass_rust

    n_split = 0
    for fn in nc.m.functions:
        for blk in fn.blocks:
            out = []
            for inst in blk.instructions:
                si = inst.sync_info
                if si is not None and len(si.on_wait) > max_waits:
                    waits = list(si.on_wait)
                    for i in range(max_waits, len(waits), max_waits):
                        nop = mybir.InstNoOp(
                            name=f"I-mw{nc.next_id()}", ins=[], outs=[])
                        nop.engine = inst.engine
                        nop.sync_info = bass_rust.SyncInfo(
                            on_wait=waits[i:i + max_waits], on_update=[])
                        out.append(nop)
                    si.on_wait = waits[:max_waits]
                    n_split += 1
                out.append(inst)
            blk.instructions = out
    return n_split


# ---------------------------------------------------------------------------
# Config
# ---------------------------------------------------------------------------
class Config:
    def __init__(self, N=50000, E=800000, F=64, H=4, n_cores=8):
        self.N, self.E, self.F, self.H, self.n_cores = N, E, F, H, n_cores
        self.HF = H * F                      # 256
        assert N % n_cores == 0
        self.NPC = N // n_cores              # own nodes per core
        self.NBLK = math.ceil(self.NPC / BW)  # dst blocks per core (127 wide)
        self.lastw = self.NPC - BW * (self.NBLK - 1)
        self.NTO = math.ceil(self.NPC / P)   # own-node transform tiles (128)
        self.NPAD = self.NTO * P             # transform-padded own rows
        self.NPAD2 = self.NBLK * BW + P      # block-read padded own rows
        self.FC_IN = 3 * F + 1               # 193
        self.FC_HID = self.FC_IN // 2        # 96
        self.OUT = 10


# ---------------------------------------------------------------------------
# Host-side prep
# ---------------------------------------------------------------------------
def host_prep(cfg, inputs):
    N, E, H, F, HF = cfg.N, cfg.E, cfg.H, cfg.F, cfg.HF
    NPC, NBLK = cfg.NPC, cfg.NBLK

    x = np.asarray(inputs["x"], np.float32)
    src = np.asarray(inputs["edge_index"][0], np.int64).astype(np.int32)
    dst = np.asarray(inputs["edge_index"][1], np.int64).astype(np.int32)
    ea = np.asarray(inputs["edge_attr"], np.float32).reshape(-1)

    order = np.argsort(dst, kind="stable")
    src_s, dst_s, ea_s = src[order], dst[order], ea[order]

    core_of = dst_s // NPC
    blk_of = (dst_s - core_of * NPC) // BW
    key = core_of * NBLK + blk_of
    starts = np.zeros(cfg.n_cores * NBLK + 1, np.int64)
    np.cumsum(np.bincount(key, minlength=cfg.n_cores * NBLK), out=starts[1:])
    counts = starts[1:] - starts[:-1]
    tblk = max(1, int(np.ceil(counts / P).max()))

    glw = np.asarray(inputs["glw"], np.float32)
    glb = np.asarray(inputs["glb"], np.float32)
    grw = np.asarray(inputs["grw"], np.float32)
    grb = np.asarray(inputs["grb"], np.float32)
    gew = np.asarray(inputs["gew"], np.float32)
    gatt = np.asarray(inputs["gatt"], np.float32)
    gbias = np.asarray(inputs["gbias"], np.float32)
    W1 = np.asarray(inputs["W1"], np.float32)
    b1 = np.asarray(inputs["b1"], np.float32)
    W2 = np.asarray(inputs["W2"], np.float32)
    b2 = np.asarray(inputs["b2"], np.float32)
    W3 = np.asarray(inputs["W3"], np.float32)
    b3 = np.asarray(inputs["b3"], np.float32)
    pt = np.asarray(inputs["problemType"], np.float32).reshape(1)

    W1_aug = np.concatenate([W1, b1[None, :]], 0)
    W1a = np.ascontiguousarray(W1_aug[:P])
    W1b = np.ascontiguousarray(W1_aug[P:])
    W2_aug = np.concatenate([W2, b2[None, :]], 0)
    W3_aug = np.concatenate([W3, b3[None, :]], 0)

    has_bias = [bool(np.any(glb)) or bool(np.any(grb)), bool(np.any(gbias))]

    iota_h = np.tile(np.arange(P, dtype=np.float16)[None, :], (P, G))
    ident_h = np.eye(P, dtype=np.float16)
    shared = dict(
        W1a=W1a, W1b=W1b, W2_aug=W2_aug, W3_aug=W3_aug,
        g_tail=np.array([[pt[0]], [1.0]], np.float32),
        iota_in=iota_h, ident_in=ident_h,
    )
    for i in range(2):
        shared[f"Wl{i}"] = np.ascontiguousarray(glw[i]).astype(np.float16)
        shared[f"bl{i}"] = glb[i].reshape(1, HF).astype(np.float16)
        shared[f"Wr{i}"] = np.ascontiguousarray(grw[i]).astype(np.float16)
        shared[f"br{i}"] = grb[i].reshape(1, HF).astype(np.float16)
        shared[f"We{i}"] = np.tile(gew[i].reshape(1, HF), (1, G)).astype(np.float16)
        shared[f"att{i}"] = np.tile(gatt[i].reshape(1, HF), (1, G)).astype(np.float16)
        shared[f"gb{i}"] = gbias[i].reshape(1, F)

    in_maps = []
    for c in range(cfg.n_cores):
        meta_i = np.zeros((NBLK, P, tblk), np.int32)
        meta_h = np.zeros((NBLK, P, 2 * tblk), np.float16)
        meta_h[:, :, :tblk] = -1.0  # dst_local = -1 -> dead lane
        for b in range(NBLK):
            s0, s1 = starts[c * NBLK + b], starts[c * NBLK + b + 1]
            cnt = s1 - s0
            if cnt == 0:
                continue
            es, ed, eea = src_s[s0:s1], dst_s[s0:s1], ea_s[s0:s1]
            ntile = math.ceil(cnt / P)
            pad = ntile * P - cnt
            es_p = np.concatenate([es, np.zeros(pad, np.int32)])
            dl_p = np.concatenate(
                [(ed - (c * NPC + b * BW)).astype(np.float16),
                 np.full(pad, -1.0, np.float16)])
            ea_p = np.concatenate([eea.astype(np.float16),
                                   np.zeros(pad, np.float16)])
            meta_i[b, :, :ntile] = es_p.reshape(ntile, P).T
            meta_h[b, :, :ntile] = dl_p.reshape(ntile, P).T
            meta_h[b, :, tblk:tblk + ntile] = ea_p.reshape(ntile, P).T

        blkw = np.minimum(BW, NPC - np.arange(NBLK) * BW)
        lane = np.arange(P)
        selfdstl = np.where(lane[:, None] < blkw[None, :],
                            lane[:, None].astype(np.float16),
                            np.float16(-1.0))
        selfdstl = (selfdstl + np.zeros((1, NBLK), np.float16)).astype(np.float16)

        x_own = np.zeros((cfg.NPAD2, F), np.float16)
        x_own[:NPC] = x[c * NPC:(c + 1) * NPC].astype(np.float16)
        xT_own = np.zeros((F, cfg.NPAD), np.float16)
        xT_own[:, :NPC] = x[c * NPC:(c + 1) * NPC].T.astype(np.float16)

        m = dict(shared)
        m.update(meta_i=meta_i, meta_h=meta_h, selfdstl=selfdstl,
                 x_own=x_own, xT_own=xT_own)
        in_maps.append(m)

    return in_maps, tblk, has_bias


# ---------------------------------------------------------------------------
# Bass program builder
# ---------------------------------------------------------------------------
def build(cfg, tblk, has_bias, split=True):
    N, F, H, HF = cfg.N, cfg.F, cfg.H, cfg.HF
    NPC, NBLK, NPAD, NPAD2 = cfg.NPC, cfg.NBLK, cfg.NPAD, cfg.NPAD2

    nc = bass.Bass("TRN2", target_bir_lowering=False, debug=False,
                   num_devices=cfg.n_cores)

    def din(name, shape, dt=F32):
        return nc.dram_tensor(name, list(shape), dt, kind="ExternalInput").ap()

    xT_own = din("xT_own", (F, NPAD), F16)
    x_own = din("x_own", (NPAD2, F), F16)
    meta_i = din("meta_i", (NBLK, P, tblk), I32)
    meta_h = din("meta_h", (NBLK, P, 2 * tblk), F16)
    selfdstl = din("selfdstl", (P, NBLK), F16)
    Wl = [din(f"Wl{i}", (F, HF), F16) for i in range(2)]
    bl = [din(f"bl{i}", (1, HF), F16) for i in range(2)]
    Wr = [din(f"Wr{i}", (F, HF), F16) for i in range(2)]
    br = [din(f"br{i}", (1, HF), F16) for i in range(2)]
    We = [din(f"We{i}", (1, G * HF), F16) for i in range(2)]
    att = [din(f"att{i}", (1, G * HF), F16) for i in range(2)]
    gb = [din(f"gb{i}", (1, F)) for i in range(2)]
    W1a = din("W1a", (P, cfg.FC_HID))
    W1b = din("W1b", (cfg.FC_IN + 1 - P, cfg.FC_HID))
    W2_aug = din("W2_aug", (cfg.FC_HID + 1, cfg.FC_HID))
    W3_aug = din("W3_aug", (cfg.FC_HID + 1, cfg.OUT))
    g_tail = din("g_tail", (2, 1))
    iota_in = din("iota_in", (P, G * P), F16)
    ident_in = din("ident_in", (P, P), F16)

    out_t = nc.dram_tensor("out", [1, cfg.OUT], F32, kind="ExternalOutput").ap()

    xl_t = nc.dram_tensor("xl_t", [N, HF], F16, addr_space="Shared").ap()
    xl_own = nc.dram_tensor("xl_own", [NPAD2, HF], F16).ap()
    xr_own = nc.dram_tensor("xr_own", [NPAD2, HF], F16).ap()
    x1_own = nc.dram_tensor("x1_own", [NPAD2, F], F16).ap()

    with tile.TileContext(nc) as tc:
        with (
            tc.tile_pool(name="pers", bufs=1) as pers,
            tc.tile_pool(name="dram", bufs=1, space="DRAM") as drp,
        ):
            iota_h = pers.tile([P, G * P], F16, tag="iota_h")
            nc.sync.dma_start(out=iota_h[:], in_=iota_in[:, :])
            identity_h = pers.tile([P, P], F16, tag="identity_h")
            nc.sync.dma_start(out=identity_h[:], in_=ident_in[:, :])
            ones_col_h = pers.tile([P, 1], F16, tag="ones_col_h")
            nc.vector.memset(ones_col_h[:], 1.0)
            ones_colG_h = pers.tile([P, G], F16, tag="ones_colG_h")
            nc.vector.memset(ones_colG_h[:], 1.0)
            ones_row_h = pers.tile([1, P], F16, tag="ones_row_h")
            nc.vector.memset(ones_row_h[:], 1.0)
            ones_row_f = pers.tile([1, P], F32, tag="ones_row_f")
            nc.vector.memset(ones_row_f[:], 1.0)
            la_all = pers.tile([P, NBLK], F16, tag="la_all")
            nc.vector.memset(la_all[:], 0.0)
            sums_sb = pers.tile([F, 3], F32, tag="sums_sb")
            nc.vector.memset(sums_sb[:], 0.0)
            selfdstl_sb = pers.tile([P, NBLK], F16, tag="selfdstl_sb")
            nc.sync.dma_start(out=selfdstl_sb[:], in_=selfdstl[:, :])

            # zero the padded tails of the own tables once (dead lanes are
            # multiplied by zero, but NaN*0 would poison PSUM)
            zpad = pers.tile([P, HF], F16, tag="zpad")
            nc.vector.memset(zpad[:], 0.0)
            r = NPAD
            while r < NPAD2:
                w = min(P, NPAD2 - r)
                nc.sync.dma_start(out=xl_own[r:r + w, :], in_=zpad[:w, :])
                nc.sync.dma_start(out=xr_own[r:r + w, :], in_=zpad[:w, :])
                r += w
            r = NPC
            while r < NPAD2:
                w = min(P, NPAD2 - r)
                nc.sync.dma_start(out=x1_own[r:r + w, :], in_=zpad[:w, :F])
                r += w

            ar_in = drp.tile([F, 3], F32, tag="ar_in")
            ar_out = drp.tile([F, 3], F32, tag="ar_out")

            for l in range(2):
                _transforms(cfg, nc, tc, l, xT_own, x1_own, Wl[l], bl[l],
                            Wr[l], br[l], xl_t, xl_own, xr_own,
                            identity_h, ones_row_h, has_bias[0])
                _edge_pass(cfg, nc, tc, l, tblk, meta_i, meta_h,
                           selfdstl_sb, We[l], att[l], gb[l],
                           xl_t, xl_own, xr_own, x_own, x1_own,
                           la_all, sums_sb, iota_h, identity_h,
                           ones_row_h, ones_row_f, ones_col_h, ones_colG_h,
                           has_bias[1])

            _head(cfg, nc, tc, sums_sb, ar_in, ar_out, W1a, W1b,
                  W2_aug, W3_aug, g_tail, out_t)

    if split:
        split_multi_waits(nc)
    return nc


def _transforms(cfg, nc, tc, l, xT_own, x1_own, Wl, bl, Wr, br,
                xl_t, xl_own, xr_own, identity_h, ones_row_h, has_bias):
    """Own-shard transforms xl_own / xr_own, then AllGather -> xl_t."""
    F, HF, NTO, NPC = cfg.F, cfg.HF, cfg.NTO, cfg.NPC
    with (
        tc.tile_pool(name=f"tf{l}", bufs=4) as tfp,
        tc.tile_pool(name=f"tfw{l}", bufs=1) as twp,
        tc.tile_pool(name=f"tfps{l}", bufs=3, space="PSUM") as tps,
    ):
        Wl_sb = twp.tile([F, HF], F16, tag="Wl_sb")
        nc.sync.dma_start(out=Wl_sb[:], in_=Wl[:, :])
        Wr_sb = twp.tile([F, HF], F16, tag="Wr_sb")
        nc.sync.dma_start(out=Wr_sb[:], in_=Wr[:, :])
        bl_s = br_s = None
        if has_bias:
            bl_s = twp.tile([1, HF], F16, tag="bl_sb")
            nc.sync.dma_start(out=bl_s[:], in_=bl[:, :])
            br_s = twp.tile([1, HF], F16, tag="br_sb")
            nc.sync.dma_start(out=br_s[:], in_=br[:, :])

        for t in range(NTO):
            r0 = t * P
            if l == 0:
                lhs = tfp.tile([F, P], F16, tag="lhs")
                nc.sync.dma_start(out=lhs[:], in_=xT_own[:, r0:r0 + P])
            else:
                xin = tfp.tile([P, F], F16, tag="xin")
                nc.sync.dma_start(out=xin[:], in_=x1_own[r0:r0 + P, :])
                ps_tr = tps.tile([F, P], F16, tag="ps_tr")
                nc.tensor.transpose(out=ps_tr[:], in_=xin[:],
                                    identity=identity_h[:])
                lhs = tfp.tile([F, P], F16, tag="lhs")
                nc.vector.tensor_copy(lhs[:], ps_tr[:])

            for (W_sb, b_sb, dstt) in ((Wl_sb, bl_s, xl_own),
                                       (Wr_sb, br_s, xr_own)):
                ps = tps.tile([P, HF], F32, tag="ps_tf")
                nc.tensor.matmul(out=ps[:], lhsT=lhs[:], rhs=W_sb[:],
                                 start=True, stop=not has_bias)
                if has_bias:
                    nc.tensor.matmul(out=ps[:], lhsT=ones_row_h[:],
                                     rhs=b_sb[:], start=False, stop=True)
                so = tfp.tile([P, HF], F16, tag="so")
                nc.scalar.copy(so[:], ps[:])
                nc.sync.dma_start(out=dstt[r0:r0 + P, :], in_=so[:])

    nc.gpsimd.collective_compute(
        "AllGather", mybir.AluOpType.bypass,
        replica_groups=[list(range(cfg.n_cores))],
        ins=[xl_own[0:NPC, :]], outs=[xl_t[:, :]])


def _edge_pass(cfg, nc, tc, l, tblk, meta_i, meta_h, selfdstl_sb, We, att, gb,
               xl_t, xl_own, xr_own, x_own, x1_own, la_all, sums_sb,
               iota_h, identity_h, ones_row_h, ones_row_f, ones_col_h,
               ones_colG_h, has_gbias):
    N, F, H, HF = cfg.N, cfg.F, cfg.H, cfg.HF
    NBLK = cfg.NBLK
    VC = HF + H + 2  # per-tile vals columns: [weighted(256) | p(4) | ea | one]
    n_chunks = math.ceil(tblk / G)

    with (
        tc.tile_pool(name=f"eb{l}", bufs=1) as ebp,
        tc.tile_pool(name=f"ed{l}", bufs=3) as edp,
        tc.tile_pool(name=f"eg{l}", bufs=3) as egp,
        tc.tile_pool(name=f"em{l}", bufs=2) as emp,
        tc.tile_pool(name=f"eps{l}", bufs=2, space="PSUM") as eps,
        tc.tile_pool(name=f"ebb{l}", bufs=1, space="PSUM") as bps,
        tc.tile_pool(name=f"etr{l}", bufs=2, space="PSUM") as trs,
        tc.tile_pool(name=f"esp{l}", bufs=2, space="PSUM") as sps,
    ):
        def bcast_h(row_ap, width, tag):
            t = ebp.tile([P, width], F16, tag=tag)
            for off in range(0, width, 512):
                w = min(512, width - off)
                ps = sps.tile([P, 512], F32, tag="ps_bc")
                nc.tensor.matmul(out=ps[:, :w], lhsT=ones_row_h[:],
                                 rhs=row_ap[:, off:off + w],
                                 start=True, stop=True)
                nc.scalar.copy(t[:, off:off + w], ps[:, :w])
            return t

        att_r = ebp.tile([1, G * HF], F16, tag="att_r")
        nc.sync.dma_start(out=att_r[:], in_=att[:, :])
        att_bc = bcast_h(att_r, G * HF, "att_bc")
        gb_bc = None
        if has_gbias:
            gb_r = ebp.tile([1, F], F32, tag="gb_r")
            nc.sync.dma_start(out=gb_r[:], in_=gb[:, :])
            ps_gb = sps.tile([P, 512], F32, tag="ps_bc")
            nc.tensor.matmul(out=ps_gb[:, :F], lhsT=ones_row_f[:], rhs=gb_r[:],
                             start=True, stop=True)
            gb_bc = ebp.tile([P, F], F32, tag="gb_bc")
            nc.scalar.copy(gb_bc[:], ps_gb[:, :F])

        x_src = x_own if l == 0 else x1_own

        for b in range(NBLK):
            mi = emp.tile([P, tblk], I32, tag="mi")
            nc.sync.dma_start(out=mi[:], in_=meta_i[b, :, :])
            mh = emp.tile([P, 2 * tblk], F16, tag="mh")
            nc.sync.dma_start(out=mh[:], in_=meta_h[b, :, :])

            # [xr rows of this 127-node block ; We row]
            xr_aug = egp.tile([P, HF], F16, tag="xr_aug")
            nc.sync.dma_start(out=xr_aug[:BW, :],
                              in_=xr_own[b * BW:b * BW + BW, :])
            nc.sync.dma_start(out=xr_aug[BW:P, :], in_=We[0:1, 0:HF])

            xl_self = edp.tile([P, HF], F16, tag="xl_self")
            nc.vector.memset(xl_self[:], 0.0)
            nc.sync.dma_start(out=xl_self[:BW, :],
                              in_=xl_own[b * BW:b * BW + BW, :])

            psb = eps.tile([P, VC], F32, tag="psb")

            def chunk(xl_ap, dl_ap, ea_ap, g, first, last, is_self=False):
                gHF = g * HF
                # S_aug: one-hot over 127 dst-locals, col 127 = ea
                S = edp.tile([P, G * P], F16, tag="S")
                nc.vector.tensor_tensor(
                    out=S[:, :g * P].rearrange("p (g n) -> p g n", n=P),
                    in0=iota_h[:, :g * P].rearrange("p (g n) -> p g n", n=P),
                    in1=dl_ap.rearrange("p (g o) -> p g o", o=1)
                        .to_broadcast([P, g, P]),
                    op=mybir.AluOpType.is_equal)
                for j in range(g):
                    nc.scalar.copy(S[:, j * P + BW:j * P + P],
                                   ea_ap[:, j:j + 1])

                # per tile: transpose S_aug, then xr_dst + ea*We via matmul
                psum_b = bps.tile([P, G * HF], F32, tag="psum_b")
                STs = []
                for j in range(g):
                    ps_tr = trs.tile([P, P], F16, tag="ps_str")
                    nc.tensor.transpose(out=ps_tr[:],
                                        in_=S[:, j * P:(j + 1) * P],
                                        identity=identity_h[:])
                    ST = edp.tile([P, P], F16, tag="ST")
                    nc.scalar.copy(ST[:], ps_tr[:])
                    STs.append(ST)
                    nc.tensor.matmul(out=psum_b[:, j * HF:(j + 1) * HF],
                                     lhsT=ST[:], rhs=xr_aug[:],
                                     start=True, stop=True)

                # b = xl_s + (xr_d + ea*We) ; m = leakyrelu(b, 0.2)
                b_sb = edp.tile([P, G * HF], F16, tag="b_sb")
                nc.vector.tensor_tensor(out=b_sb[:, :gHF],
                                        in0=xl_ap[:, :gHF],
                                        in1=psum_b[:, :gHF],
                                        op=mybir.AluOpType.add)
                m_sb = edp.tile([P, G * HF], F16, tag="m_sb")
                nc.scalar.activation(m_sb[:, :gHF], b_sb[:, :gHF],
                                     mybir.ActivationFunctionType.Prelu,
                                     alpha=0.2)
                lm = edp.tile([P, G * HF], F16, tag="lm")
                nc.vector.tensor_tensor(out=lm[:, :gHF], in0=m_sb[:, :gHF],
                                        in1=att_bc[:, :gHF],
                                        op=mybir.AluOpType.mult)
                pl = edp.tile([P, G * H], F16, tag="pl")
                with nc.allow_low_precision(reason="fp16 edge logits"):
                    nc.vector.tensor_reduce(
                        out=pl[:, :g * H],
                        in_=lm[:, :gHF].rearrange("p (a f) -> p a f", f=F),
                        op=mybir.AluOpType.add, axis=mybir.AxisListType.X)

                vals = edp.tile([P, G * VC], F16, tag="vals")
                v3 = vals[:, :g * VC].rearrange("p (g c) -> p g c", c=VC)
                nc.scalar.activation(
                    v3[:, :, HF:HF + H],
                    pl[:, :g * H].rearrange("p (g h) -> p g h", h=H),
                    mybir.ActivationFunctionType.Exp)
                nc.scalar.copy(v3[:, :, HF + H:HF + H + 1],
                               ea_ap.rearrange("p (g o) -> p g o", o=1))
                nc.scalar.copy(v3[:, :, HF + H + 1:HF + H + 2],
                               ones_colG_h[:, :g]
                               .rearrange("p (g o) -> p g o", o=1))
                nc.vector.tensor_tensor(
                    out=v3[:, :, 0:HF].rearrange("p g (h f) -> p g h f", f=F),
                    in0=xl_ap.rearrange("p (g h f) -> p g h f", h=H, f=F),
                    in1=v3[:, :, HF:HF + H]
                        .rearrange("p g (h o) -> p g h o", o=1)
                        .to_broadcast([P, g, H, F]),
                    op=mybir.AluOpType.mult)

                for j in range(g):
                    nc.tensor.matmul(
                        out=psb[:BW, :],
                        lhsT=S[:, j * P:j * P + BW],
                        rhs=vals[:, j * VC:(j + 1) * VC],
                        start=(first and j == 0),
                        stop=(last and j == g - 1),
                        skip_group_check=is_self)

            for ci in range(n_chunks):
                k0 = ci * G
                g = min(G, tblk - k0)
                xl_slab = egp.tile([P, G * HF], F16, tag="xl_slab")
                for j in range(g):
                    nc.gpsimd.indirect_dma_start(
                        out=xl_slab[:, j * HF:(j + 1) * HF], out_offset=None,
                        in_=xl_t[:, :],
                        in_offset=bass.IndirectOffsetOnAxis(
                            ap=mi[:, k0 + j:k0 + j + 1], axis=0))
                chunk(xl_slab[:, :g * HF],
                      mh[:, k0:k0 + g], mh[:, tblk + k0:tblk + k0 + g],
                      g, first=(ci == 0), last=(ci == n_chunks - 1))

            # loop_attr for this block (reads psum before the self-loop tile)
            deg1 = edp.tile([P, 1], F32, tag="deg1")
            nc.vector.tensor_scalar(out=deg1[:BW], in0=psb[:BW, VC - 1:VC],
                                    scalar1=1.0, scalar2=None,
                                    op0=mybir.AluOpType.max)
            rec1 = edp.tile([P, 1], F32, tag="rec1")
            nc.vector.reciprocal(rec1[:BW], deg1[:BW])
            nc.vector.tensor_scalar(out=la_all[:BW, b:b + 1],
                                    in0=psb[:BW, VC - 2:VC - 1],
                                    scalar1=rec1[:BW, 0:1], scalar2=None,
                                    op0=mybir.AluOpType.mult)

            # self-loop tile (xl rows loaded directly; xr via the S_aug path)
            chunk(xl_self[:, :HF], selfdstl_sb[:, b:b + 1],
                  la_all[:, b:b + 1], 1, first=False, last=True, is_self=True)

            # ---- block epilogue (fp32, on 127 rows) ----
            blkw = BW if b < NBLK - 1 else cfg.lastw
            d4 = edp.tile([P, H], F32, tag="d4")
            nc.vector.tensor_scalar(out=d4[:BW], in0=psb[:BW, HF:HF + H],
                                    scalar1=float(H), scalar2=1e-30,
                                    op0=mybir.AluOpType.mult,
                                    op1=mybir.AluOpType.max)
            rec4 = edp.tile([P, H], F32, tag="rec4")
            nc.vector.reciprocal(rec4[:BW], d4[:BW])
            hm = edp.tile([P, F], F32, tag="hm")
            tmp64 = edp.tile([P, F], F32, tag="tmp64")
            for h in range(H):
                dsth = hm if h == 0 else tmp64
                nc.vector.tensor_scalar(out=dsth[:BW],
                                        in0=psb[:BW, h * F:(h + 1) * F],
                                        scalar1=rec4[:BW, h:h + 1],
                                        scalar2=None,
                                        op0=mybir.AluOpType.mult)
                if h > 0:
                    nc.vector.tensor_tensor(out=hm[:BW], in0=hm[:BW],
                                            in1=tmp64[:BW],
                                            op=mybir.AluOpType.add)
            u = hm
            if has_gbias:
                u = edp.tile([P, F], F32, tag="u")
                nc.vector.tensor_tensor(out=u[:BW], in0=hm[:BW],
                                        in1=gb_bc[:BW],
                                        op=mybir.AluOpType.add)
            v = edp.tile([P, F], F32, tag="v")
            nc.scalar.activation(v[:BW], u[:BW],
                                 mybir.ActivationFunctionType.Prelu,
                                 alpha=0.01)
            xo = edp.tile([P, F], F16, tag="xo")
            nc.sync.dma_start(out=xo[:BW], in_=x_src[b * BW:b * BW + BW, :])
            xof = edp.tile([P, F], F32, tag="xof")
            nc.vector.tensor_copy(xof[:BW], xo[:BW])
            xn = edp.tile([P, F], F32, tag="xn")
            nc.vector.tensor_tensor(out=xn[:BW], in0=xof[:BW], in1=v[:BW],
                                    op=mybir.AluOpType.add)
            xnh = edp.tile([P, F], F16, tag="xnh")
            nc.vector.tensor_copy(xnh[:BW], xn[:BW])
            if l == 0:
                nc.sync.dma_start(out=x1_own[b * BW:b * BW + blkw, :],
                                  in_=xnh[:blkw])

            def colsum(src_tile, col):
                pcs = sps.tile([F, 1], F32, tag="ps_bc")
                nc.tensor.matmul(out=pcs[:], lhsT=src_tile[:blkw, :],
                                 rhs=ones_col_h[:blkw, :], start=True,
                                 stop=True)
                nc.vector.tensor_tensor(out=sums_sb[:, col:col + 1],
                                        in0=sums_sb[:, col:col + 1],
                                        in1=pcs[:],
                                        op=mybir.AluOpType.add)

            if l == 0:
                colsum(xo, 0)
                colsum(xnh, 1)
            else:
                colsum(xnh, 2)


def _head(cfg, nc, tc, sums_sb, ar_in, ar_out, W1a, W1b, W2_aug, W3_aug,
          g_tail, out_t):
    F, FH, OUT = cfg.F, cfg.FC_HID, cfg.OUT
    n_w1b = cfg.FC_IN + 1 - P  # 66
    inv_n = 1.0 / cfg.N
    with (
        tc.tile_pool(name="hd", bufs=1) as hd,
        tc.tile_pool(name="hdps", bufs=1, space="PSUM") as hps,
    ):
        s_loc = hd.tile([F, 3], F32, tag="s_loc")
        nc.vector.tensor_copy(s_loc[:], sums_sb[:])
        nc.sync.dma_start(out=ar_in[:, :], in_=s_loc[:])
        nc.gpsimd.collective_compute(
            "AllReduce", mybir.AluOpType.add,
            replica_groups=[list(range(cfg.n_cores))],
            ins=[ar_in.opt()], outs=[ar_out.opt()])
        s_red = hd.tile([F, 3], F32, tag="s_red")
        nc.sync.dma_start(out=s_red[:], in_=ar_out[:, :])

        g_a = hd.tile([P, 1], F32, tag="g_a")
        g_b = hd.tile([n_w1b, 1], F32, tag="g_b")
        nc.scalar.mul(g_a[0:F, :], s_red[:, 0:1], inv_n)
        nc.scalar.mul(g_a[F:2 * F, :], s_red[:, 1:2], inv_n)
        nc.scalar.mul(g_b[0:F, :], s_red[:, 2:3], inv_n)
        nc.sync.dma_start(out=g_b[F:F + 2, :], in_=g_tail[:, :])

        W1a_sb = hd.tile([P, FH], F32, tag="W1a_sb")
        nc.sync.dma_start(out=W1a_sb[:], in_=W1a[:, :])
        W1b_sb = hd.tile([n_w1b, FH], F32, tag="W1b_sb")
        nc.sync.dma_start(out=W1b_sb[:], in_=W1b[:, :])
        W2_sb = hd.tile([FH + 1, FH], F32, tag="W2_sb")
        nc.sync.dma_start(out=W2_sb[:], in_=W2_aug[:, :])
        W3_sb = hd.tile([FH + 1, OUT], F32, tag="W3_sb")
        nc.sync.dma_start(out=W3_sb[:], in_=W3_aug[:, :])

        h1p = hps.tile([FH, 1], F32, tag="h1p")
        nc.tensor.matmul(out=h1p[:], lhsT=W1a_sb[:], rhs=g_a[:],
                         start=True, stop=False)
        nc.tensor.matmul(out=h1p[:], lhsT=W1b_sb[:], rhs=g_b[:],
                         start=False, stop=True)
        h1s = hd.tile([FH + 1, 1], F32, tag="h1s")
        nc.scalar.activation(h1s[0:FH, :], h1p[:],
                             mybir.ActivationFunctionType.Prelu, alpha=0.01)
        nc.vector.memset(h1s[FH:FH + 1, :], 1.0)

        h2p = hps.tile([FH, 1], F32, tag="h2p")
        nc.tensor.matmul(out=h2p[:], lhsT=W2_sb[:], rhs=h1s[:],
                         start=True, stop=True)
        h2s = hd.tile([FH + 1, 1], F32, tag="h2s")
        nc.scalar.activation(h2s[0:FH, :], h2p[:],
                             mybir.ActivationFunctionType.Prelu, alpha=0.01)
        nc.vector.memset(h2s[FH:FH + 1, :], 1.0)

        op = hps.tile([OUT, 1], F32, tag="op")
        nc.tensor.matmul(out=op[:], lhsT=W3_sb[:], rhs=h2s[:],
                         start=True, stop=True)
        o_sb = hd.tile([OUT, 1], F32, tag="o_sb")
        nc.vector.tensor_copy(o_sb[:], op[:])
        nc.sync.dma_start(out=out_t[0:1, :].rearrange("a b -> b a"),
                          in_=o_sb[:])


# ---------------------------------------------------------------------------
# Entry point
# ---------------------------------------------------------------------------
def kernel(**inputs):
    apply_tile_patch()
    from concourse.bass_utils import run_bass_kernel_spmd

    cfg = Config()
    in_maps, tblk, has_bias = host_prep(cfg, inputs)
    nc = build(cfg, tblk, has_bias)
    res = run_bass_kernel_spmd(nc, in_maps, list(range(cfg.n_cores)))
    return np.asarray(res.results[0]["out"], np.float32)
